# revision 32
# baseline (speedup 1.0000x reference)
"""BiDAF attention kernel for trn2 (8 NeuronCores, pure data parallel).

v2: instruction-count-optimized.  S^T layout (LP on partitions) so both
softmaxes reduce along the free axis.  Aq = w^T q^T precomputed for all 16
batches in batched N=400 matmuls; h computed with reversed operands
(lhsT = eq column, rhs = p rows) with the normalizing sum fused in via a
ones-column appended to p; output rows assembled in SBUF so each LP-chunk
is a single contiguous-row DMA.
"""

from contextlib import ExitStack

import numpy as np

import concourse.bass as bass
import concourse.mybir as mybir
import concourse.tile as tile
from concourse.bass_utils import run_bass_kernel_spmd
from concourse.masks import make_identity

F32 = mybir.dt.float32
AX = mybir.AxisListType
ALU = mybir.AluOpType
ACTF = mybir.ActivationFunctionType

B, LP, LQ, H = 128, 400, 100, 256
NCORES = 8
BP = B // NCORES  # batches per core
ROWS = [(0, 128), (128, 128), (256, 128), (384, 16)]
BIG = float(np.float32(3.0e38))


def build_nc():
    nc = bass.Bass("TRN2", target_bir_lowering=False, debug=False)

    pn = nc.dram_tensor("pn", [BP, LP, H], F32, kind="ExternalInput")
    pt = nc.dram_tensor("pt", [BP, H, LP], F32, kind="ExternalInput")
    qn = nc.dram_tensor("qn", [BP, LQ, H], F32, kind="ExternalInput")
    qt = nc.dram_tensor("qt", [BP, H, LQ], F32, kind="ExternalInput")
    w = nc.dram_tensor("w", [H, H], F32, kind="ExternalInput")
    g = nc.dram_tensor("g", [BP, LP, 4 * H], F32, kind="ExternalOutput")

    with tile.TileContext(nc) as tc, ExitStack() as ctx:
        cpool = ctx.enter_context(tc.tile_pool(name="consts", bufs=1))
        wp = ctx.enter_context(tc.tile_pool(name="work", bufs=4))
        ppb = ctx.enter_context(tc.tile_pool(name="psb", bufs=2, space="PSUM"))
        pps = ctx.enter_context(tc.tile_pool(name="pss", bufs=2, space="PSUM"))
        ppu = ctx.enter_context(tc.tile_pool(name="psu", bufs=2, space="PSUM"))
        ppt = ctx.enter_context(tc.tile_pool(name="pst", bufs=2, space="PSUM"))

        # ---- constants ----
        ident = cpool.tile([128, 128], F32)
        make_identity(nc, ident[:])
        ones_r = cpool.tile([1, 128], F32)
        nc.vector.memset(ones_r[:], 1.0)
        Wt = cpool.tile([128, 2, H], F32)
        nc.sync.dma_start(Wt[:, 0, :], w[0:128, :])
        nc.sync.dma_start(Wt[:, 1, :], w[128:256, :])

        # ---- pre-pass: Aq = w^T q^T for ALL batches, batched 4-wide ----
        QtA = cpool.tile([128, 2, BP * LQ], F32)       # (h-part, kc, b*l)
        AqA = cpool.tile([128, 2, BP * LQ], F32)
        for gi in range(BP // 4):
            for kc in range(2):
                nc.sync.dma_start(
                    QtA[:, kc, gi * 400:(gi + 1) * 400].rearrange(
                        "p (b l) -> p b l", b=4),
                    qt[gi * 4:(gi + 1) * 4,
                       kc * 128:(kc + 1) * 128, :].rearrange(
                        "b p l -> p b l"))
        for gi in range(BP // 4):
            for ms in range(2):
                psAq = ppb.tile([128, 400], F32, tag="big")
                for kc in range(2):
                    nc.tensor.matmul(
                        psAq[:],
                        Wt[:, kc, ms * 128:(ms + 1) * 128],
                        QtA[:, kc, gi * 400:(gi + 1) * 400],
                        start=(kc == 0), stop=(kc == 1),
                    )
                nc.scalar.copy(AqA[:, ms, gi * 400:(gi + 1) * 400], psAq[:])

        for b in range(BP):
            bq = b * LQ
            # ---------------- loads ----------------
            Pn = wp.tile([128, 4, H + 1], F32, tag="Pn")  # col 256 = ones
            nc.vector.memset(Pn[:, :, H:H + 1], 1.0)
            nc.sync.dma_start(
                Pn[0:128, 0:3, 0:H],
                pn[b, 0:384, :].rearrange("(i r) h -> r i h", r=128))
            nc.sync.dma_start(Pn[0:16, 3, 0:H], pn[b, 384:400, :])
            Pt = wp.tile([128, 2, LP], F32, tag="Pt")
            nc.sync.dma_start(Pt[:],
                              pt[b].rearrange("(k p) l -> p k l", p=128))
            Qn = wp.tile([128, H], F32, tag="Qn")
            nc.sync.dma_start(Qn[0:LQ, :], qn[b])

            # ---------------- S^T = p @ Aq  (LP, LQ) ----------------
            # two PSUM tiles so softmax readers of chunks 0-1 overlap the
            # PE writing chunks 2-3 (bank-overlap tracker serializes within
            # one tensor)
            psS01 = pps.tile([128, 2, LQ], F32, tag="st")
            psS23 = pps.tile([128, 2, LQ], F32, tag="st")
            psSc = [psS01, psS01, psS23, psS23]
            for i, (off, r) in enumerate(ROWS):
                for kc in range(2):
                    nc.tensor.matmul(
                        psSc[i][0:r, i % 2, :],
                        Pt[:, kc, off:off + r],
                        AqA[:, kc, bq:bq + LQ],
                        start=(kc == 0), stop=(kc == 1),
                    )

            # ---------------- C2Q softmax over LQ (free axis) ----------------
            NM = wp.tile([128, 4], F32, tag="NM")
            RS = wp.tile([128, 4], F32, tag="RS")
            nc.vector.memset(NM[:], BIG)
            nc.vector.memset(RS[:], 1.0)
            for i, (off, r) in enumerate(ROWS):
                nc.vector.tensor_reduce(
                    NM[0:r, i:i + 1], psSc[i][0:r, i % 2, :],
                    axis=AX.X, op=ALU.max, negate=True,
                )
            E = wp.tile([128, 4, LQ], F32, tag="E")
            for i, (off, r) in enumerate(ROWS):
                nc.scalar.activation(
                    E[0:r, i, :], psSc[i][0:r, i % 2, :], ACTF.Exp,
                    bias=NM[0:r, i:i + 1], accum_out=RS[0:r, i:i + 1],
                )
            RCP = wp.tile([128, 4], F32, tag="RCP")
            nc.vector.reciprocal(RCP[:], RS[:])
            AT = wp.tile([128, 4, LQ], F32, tag="AT")
            for i, (off, r) in enumerate(ROWS):
                nc.vector.tensor_scalar_mul(
                    AT[0:r, i, :], E[0:r, i, :], RCP[0:r, i:i + 1])

            # ---------------- Q2C ----------------
            nmn = wp.tile([128, 1], F32, tag="nmn")
            nc.vector.tensor_reduce(nmn[:], NM[:], axis=AX.X, op=ALU.min)
            psTB = ppt.tile([128, 257], F32, tag="tiny")
            nc.tensor.transpose(psTB[0:1, 0:128], nmn[:], ident[:])
            ngmin = wp.tile([1, 1], F32, tag="ngmin")   # = -gmax
            nc.vector.tensor_reduce(ngmin[:], psTB[0:1, 0:128], axis=AX.X,
                                    op=ALU.min)
            nc.tensor.matmul(psTB[0:128, 128:129], ones_r[:], ngmin[:],
                             start=True, stop=True)
            nb = wp.tile([128, 1], F32, tag="nb")
            nc.scalar.copy(nb[:], psTB[0:128, 128:129])
            EQ = wp.tile([128, 4], F32, tag="EQ")
            nc.scalar.activation(EQ[:], NM[:], ACTF.Exp,
                                 bias=nb[:], scale=-1.0)
            # ---------------- transpose a^T -> a ----------------
            psAm = ppb.tile([128, 4, 128], F32, tag="big")
            for i, (off, r) in enumerate(ROWS):
                nc.tensor.transpose(
                    psAm[0:LQ, i, 0:r], AT[0:r, i, :], ident[0:r, 0:r])
            Am = wp.tile([128, 4, 128], F32, tag="Am")
            nc.scalar.copy(Am[0:LQ, 0:3, :], psAm[0:LQ, 0:3, :])
            nc.scalar.copy(Am[0:LQ, 3, 0:16], psAm[0:LQ, 3, 0:16])

            # ---------------- U^T + output assembly ----------------
            Gt = wp.tile([128, 4, 4 * H], F32, tag="Gt")
            # block 0: passage
            nc.vector.tensor_copy(Gt[0:128, 0:3, 0:H], Pn[0:128, 0:3, 0:H])
            nc.vector.tensor_copy(Gt[0:16, 3, 0:H], Pn[0:16, 3, 0:H])
            for i, (off, r) in enumerate(ROWS):
                psU = ppu.tile([128, H], F32, tag="u")
                nc.tensor.matmul(
                    psU[0:r, :], Am[0:LQ, i, 0:r], Qn[0:LQ, :],
                    start=True, stop=True,
                )
                # block 2: p * U^T
                nc.vector.tensor_tensor(
                    Gt[0:r, i, 2 * H:3 * H], Pn[0:r, i, 0:H], psU[0:r, :],
                    op=ALU.mult)

            # h (1, 257): cols 0:256 = unnormalized h^T, col 256 = sum(eq)
            psHr = ppt.tile([128, 257], F32, tag="tiny")
            for i, (off, r) in enumerate(ROWS):
                nc.tensor.matmul(
                    psHr[0:1, :],
                    EQ[0:r, i:i + 1],
                    Pn[0:r, i, 0:H + 1],
                    start=(i == 0), stop=(i == 3),
                )
            rq = wp.tile([1, 1], F32, tag="rq")
            nc.vector.reciprocal(rq[:], psHr[0:1, H:H + 1])
            hrow = wp.tile([1, H], F32, tag="hrow")
            nc.scalar.mul(hrow[:], psHr[0:1, 0:H], rq[:])
            psHt = ppb.tile([128, H], F32, tag="big")
            nc.tensor.matmul(psHt[:], ones_r[:], hrow[:], start=True,
                             stop=True)
            HtS = wp.tile([128, H], F32, tag="HtS")
            nc.vector.tensor_copy(HtS[:], psHt[:])
            for i, (off, r) in enumerate(ROWS):
                # block 1: tiled h
                nc.vector.tensor_copy(Gt[0:r, i, H:2 * H], HtS[0:r, :])
                # block 3: p * Ht  (gpsimd, SBUF-only operands)
                nc.gpsimd.tensor_tensor(
                    Gt[0:r, i, 3 * H:4 * H], Pn[0:r, i, 0:H], HtS[0:r, :],
                    op=ALU.mult)

            for i, (off, r) in enumerate(ROWS):
                nc.sync.dma_start(g[b, off:off + r, :], Gt[0:r, i, :])

    return nc


def legalize_waits(nc):
    """Split multi-wait instructions into single-wait NoOps + instruction.

    The TPB ISA has exactly one (wait, update) EVENTS slot per 64B
    instruction; this walrus build refuses instructions with more than one
    sync wait ("Too many sync wait commands").  Tile's scheduler emits
    vector-clock waits freely, so legalize here: excess waits move onto
    engine-queue NoOps placed immediately before the instruction.
    """
    counter = 0
    for f in nc.m.functions:
        for blk in f.blocks:
            new = []
            for inst in blk.instructions:
                si = getattr(inst, "sync_info", None)
                if si is not None and len(si.on_wait) > 1:
                    waits = list(si.on_wait)
                    assert len(si.on_update) <= 1, inst
                    for wt in waits[:-1]:
                        counter += 1
                        new.append(mybir.InstNoOp(
                            name=f"I-waitnop-{counter}",
                            engine=inst.engine,
                            sync_info=mybir.SyncInfo(on_wait=[wt],
                                                     on_update=[]),
                        ))
                    inst.sync_info = mybir.SyncInfo(
                        on_wait=[waits[-1]], on_update=list(si.on_update))
                new.append(inst)
            blk.instructions = new
    return nc


def _make_in_maps(p, q, w):
    p = np.ascontiguousarray(p, dtype=np.float32)
    q = np.ascontiguousarray(q, dtype=np.float32)
    w = np.ascontiguousarray(w, dtype=np.float32)
    in_maps = []
    for c in range(NCORES):
        sl = slice(c * BP, (c + 1) * BP)
        in_maps.append({
            "pn": p[sl],
            "pt": np.ascontiguousarray(p[sl].transpose(0, 2, 1)),
            "qn": q[sl],
            "qt": np.ascontiguousarray(q[sl].transpose(0, 2, 1)),
            "w": w,
        })
    return in_maps


def run(p, q, w, trace=False):
    nc = legalize_waits(build_nc())
    res = run_bass_kernel_spmd(
        nc, _make_in_maps(p, q, w), list(range(NCORES)), trace=trace)
    out = np.concatenate([res.results[c]["g"] for c in range(NCORES)], axis=0)
    return out, res


def kernel(p, q, w):
    out, _ = run(p, q, w, trace=False)
    return out


# revision 33
# speedup vs baseline: 1.0217x; 1.0217x over previous
"""BiDAF attention kernel for trn2 (8 NeuronCores, pure data parallel).

v2: instruction-count-optimized.  S^T layout (LP on partitions) so both
softmaxes reduce along the free axis.  Aq = w^T q^T precomputed for all 16
batches in batched N=400 matmuls; h computed with reversed operands
(lhsT = eq column, rhs = p rows) with the normalizing sum fused in via a
ones-column appended to p; output rows assembled in SBUF so each LP-chunk
is a single contiguous-row DMA.
"""

from contextlib import ExitStack

import numpy as np

import concourse.bass as bass
import concourse.mybir as mybir
import concourse.tile as tile
from concourse.bass_utils import run_bass_kernel_spmd
from concourse.masks import make_identity

F32 = mybir.dt.float32
AX = mybir.AxisListType
ALU = mybir.AluOpType
ACTF = mybir.ActivationFunctionType

B, LP, LQ, H = 128, 400, 100, 256
NCORES = 8
BP = B // NCORES  # batches per core
ROWS = [(0, 128), (128, 128), (256, 128), (384, 16)]
BIG = float(np.float32(3.0e38))


def build_nc():
    nc = bass.Bass("TRN2", target_bir_lowering=False, debug=False)

    pn = nc.dram_tensor("pn", [BP, LP, H], F32, kind="ExternalInput")
    pt = nc.dram_tensor("pt", [BP, H, LP], F32, kind="ExternalInput")
    qn = nc.dram_tensor("qn", [BP, LQ, H], F32, kind="ExternalInput")
    qt = nc.dram_tensor("qt", [BP, H, LQ], F32, kind="ExternalInput")
    w = nc.dram_tensor("w", [H, H], F32, kind="ExternalInput")
    g = nc.dram_tensor("g", [BP, LP, 4 * H], F32, kind="ExternalOutput")

    with tile.TileContext(nc) as tc, ExitStack() as ctx:
        cpool = ctx.enter_context(tc.tile_pool(name="consts", bufs=1))
        wp = ctx.enter_context(tc.tile_pool(name="work", bufs=4))
        ppb = ctx.enter_context(tc.tile_pool(name="psb", bufs=2, space="PSUM"))
        pps = ctx.enter_context(tc.tile_pool(name="pss", bufs=2, space="PSUM"))
        ppu = ctx.enter_context(tc.tile_pool(name="psu", bufs=2, space="PSUM"))
        ppt = ctx.enter_context(tc.tile_pool(name="pst", bufs=2, space="PSUM"))

        # ---- constants ----
        ident = cpool.tile([128, 128], F32)
        make_identity(nc, ident[:])
        ones_r = cpool.tile([1, 128], F32)
        nc.vector.memset(ones_r[:], 1.0)
        Wt = cpool.tile([128, 2, H], F32)
        nc.sync.dma_start(Wt[:, 0, :], w[0:128, :])
        nc.sync.dma_start(Wt[:, 1, :], w[128:256, :])

        # ---- pre-pass: Aq = w^T q^T for ALL batches, batched 4-wide ----
        QtA = cpool.tile([128, 2, BP * LQ], F32)       # (h-part, kc, b*l)
        AqA = cpool.tile([128, 2, BP * LQ], F32)
        for gi in range(BP // 4):
            for kc in range(2):
                nc.sync.dma_start(
                    QtA[:, kc, gi * 400:(gi + 1) * 400].rearrange(
                        "p (b l) -> p b l", b=4),
                    qt[gi * 4:(gi + 1) * 4,
                       kc * 128:(kc + 1) * 128, :].rearrange(
                        "b p l -> p b l"))
        for gi in range(BP // 4):
            for ms in range(2):
                psAq = ppb.tile([128, 400], F32, tag="big")
                for kc in range(2):
                    nc.tensor.matmul(
                        psAq[:],
                        Wt[:, kc, ms * 128:(ms + 1) * 128],
                        QtA[:, kc, gi * 400:(gi + 1) * 400],
                        start=(kc == 0), stop=(kc == 1),
                    )
                nc.scalar.copy(AqA[:, ms, gi * 400:(gi + 1) * 400], psAq[:])

        for b in range(BP):
            bq = b * LQ
            # ---------------- loads ----------------
            Pn = wp.tile([128, 4, H + 1], F32, tag="Pn")  # col 256 = ones
            nc.vector.memset(Pn[:, :, H:H + 1], 1.0)
            nc.sync.dma_start(
                Pn[0:128, 0:3, 0:H],
                pn[b, 0:384, :].rearrange("(i r) h -> r i h", r=128))
            nc.sync.dma_start(Pn[0:16, 3, 0:H], pn[b, 384:400, :])
            Pt = wp.tile([128, 2, LP], F32, tag="Pt")
            nc.sync.dma_start(Pt[:],
                              pt[b].rearrange("(k p) l -> p k l", p=128))
            Qn = wp.tile([128, H], F32, tag="Qn")
            nc.sync.dma_start(Qn[0:LQ, :], qn[b])

            # ---------------- S^T = p @ Aq  (LP, LQ) ----------------
            # two PSUM tiles so softmax readers of chunks 0-1 overlap the
            # PE writing chunks 2-3 (bank-overlap tracker serializes within
            # one tensor)
            psS01 = pps.tile([128, 2, LQ], F32, tag="st")
            psS23 = pps.tile([128, 2, LQ], F32, tag="st")
            psSc = [psS01, psS01, psS23, psS23]
            for i, (off, r) in enumerate(ROWS):
                for kc in range(2):
                    nc.tensor.matmul(
                        psSc[i][0:r, i % 2, :],
                        Pt[:, kc, off:off + r],
                        AqA[:, kc, bq:bq + LQ],
                        start=(kc == 0), stop=(kc == 1),
                    )

            # ---------------- C2Q softmax over LQ (free axis) ----------------
            NM = wp.tile([128, 4], F32, tag="NM")
            RS = wp.tile([128, 4], F32, tag="RS")
            nc.vector.memset(NM[:], BIG)
            nc.vector.memset(RS[:], 1.0)
            for i, (off, r) in enumerate(ROWS):
                nc.vector.tensor_reduce(
                    NM[0:r, i:i + 1], psSc[i][0:r, i % 2, :],
                    axis=AX.X, op=ALU.max, negate=True,
                )
            E = wp.tile([128, 4, LQ], F32, tag="E")
            for i, (off, r) in enumerate(ROWS):
                nc.scalar.activation(
                    E[0:r, i, :], psSc[i][0:r, i % 2, :], ACTF.Exp,
                    bias=NM[0:r, i:i + 1], accum_out=RS[0:r, i:i + 1],
                )
            RCP = wp.tile([128, 4], F32, tag="RCP")
            nc.vector.reciprocal(RCP[:], RS[:])
            AT = wp.tile([128, 4, LQ], F32, tag="AT")
            for i, (off, r) in enumerate(ROWS):
                nc.vector.tensor_scalar_mul(
                    AT[0:r, i, :], E[0:r, i, :], RCP[0:r, i:i + 1])

            # ---------------- transpose a^T -> a ----------------
            psAm = ppb.tile([128, 4, 128], F32, tag="big")
            for i, (off, r) in enumerate(ROWS):
                nc.tensor.transpose(
                    psAm[0:LQ, i, 0:r], AT[0:r, i, :], ident[0:r, 0:r])
            Am = wp.tile([128, 4, 128], F32, tag="Am")
            nc.scalar.copy(Am[0:LQ, 0:3, :], psAm[0:LQ, 0:3, :])
            nc.scalar.copy(Am[0:LQ, 3, 0:16], psAm[0:LQ, 3, 0:16])

            # ---------------- U^T + output assembly ----------------
            Gt = wp.tile([128, 4, 4 * H], F32, tag="Gt")
            # block 0: passage
            nc.vector.tensor_copy(Gt[0:128, 0:3, 0:H], Pn[0:128, 0:3, 0:H])
            nc.vector.tensor_copy(Gt[0:16, 3, 0:H], Pn[0:16, 3, 0:H])
            for i, (off, r) in enumerate(ROWS):
                psU = ppu.tile([128, H], F32, tag="u")
                nc.tensor.matmul(
                    psU[0:r, :], Am[0:LQ, i, 0:r], Qn[0:LQ, :],
                    start=True, stop=True,
                )
                # block 2: p * U^T
                nc.vector.tensor_tensor(
                    Gt[0:r, i, 2 * H:3 * H], Pn[0:r, i, 0:H], psU[0:r, :],
                    op=ALU.mult)

            # ---------------- Q2C ----------------
            nmn = wp.tile([128, 1], F32, tag="nmn")
            nc.vector.tensor_reduce(nmn[:], NM[:], axis=AX.X, op=ALU.min)
            psTB = ppt.tile([128, 257], F32, tag="tiny")
            nc.tensor.transpose(psTB[0:1, 0:128], nmn[:], ident[:])
            ngmin = wp.tile([1, 1], F32, tag="ngmin")   # = -gmax
            nc.vector.tensor_reduce(ngmin[:], psTB[0:1, 0:128], axis=AX.X,
                                    op=ALU.min)
            nc.tensor.matmul(psTB[0:128, 128:129], ones_r[:], ngmin[:],
                             start=True, stop=True)
            nb = wp.tile([128, 1], F32, tag="nb")
            nc.scalar.copy(nb[:], psTB[0:128, 128:129])
            EQ = wp.tile([128, 4], F32, tag="EQ")
            nc.scalar.activation(EQ[:], NM[:], ACTF.Exp,
                                 bias=nb[:], scale=-1.0)
            # h (1, 257): cols 0:256 = unnormalized h^T, col 256 = sum(eq)
            psHr = ppt.tile([128, 257], F32, tag="tiny")
            for i, (off, r) in enumerate(ROWS):
                nc.tensor.matmul(
                    psHr[0:1, :],
                    EQ[0:r, i:i + 1],
                    Pn[0:r, i, 0:H + 1],
                    start=(i == 0), stop=(i == 3),
                )
            rq = wp.tile([1, 1], F32, tag="rq")
            nc.vector.reciprocal(rq[:], psHr[0:1, H:H + 1])
            hrow = wp.tile([1, H], F32, tag="hrow")
            nc.scalar.mul(hrow[:], psHr[0:1, 0:H], rq[:])
            psHt = ppb.tile([128, H], F32, tag="big")
            nc.tensor.matmul(psHt[:], ones_r[:], hrow[:], start=True,
                             stop=True)
            HtS = wp.tile([128, H], F32, tag="HtS")
            nc.vector.tensor_copy(HtS[:], psHt[:])
            for i, (off, r) in enumerate(ROWS):
                # block 1: tiled h
                nc.vector.tensor_copy(Gt[0:r, i, H:2 * H], HtS[0:r, :])
                # block 3: p * Ht  (gpsimd, SBUF-only operands)
                nc.gpsimd.tensor_tensor(
                    Gt[0:r, i, 3 * H:4 * H], Pn[0:r, i, 0:H], HtS[0:r, :],
                    op=ALU.mult)

            for i, (off, r) in enumerate(ROWS):
                nc.sync.dma_start(g[b, off:off + r, :], Gt[0:r, i, :])

    return nc


def legalize_waits(nc):
    """Split multi-wait instructions into single-wait NoOps + instruction.

    The TPB ISA has exactly one (wait, update) EVENTS slot per 64B
    instruction; this walrus build refuses instructions with more than one
    sync wait ("Too many sync wait commands").  Tile's scheduler emits
    vector-clock waits freely, so legalize here: excess waits move onto
    engine-queue NoOps placed immediately before the instruction.
    """
    counter = 0
    for f in nc.m.functions:
        for blk in f.blocks:
            new = []
            for inst in blk.instructions:
                si = getattr(inst, "sync_info", None)
                if si is not None and len(si.on_wait) > 1:
                    waits = list(si.on_wait)
                    assert len(si.on_update) <= 1, inst
                    for wt in waits[:-1]:
                        counter += 1
                        new.append(mybir.InstNoOp(
                            name=f"I-waitnop-{counter}",
                            engine=inst.engine,
                            sync_info=mybir.SyncInfo(on_wait=[wt],
                                                     on_update=[]),
                        ))
                    inst.sync_info = mybir.SyncInfo(
                        on_wait=[waits[-1]], on_update=list(si.on_update))
                new.append(inst)
            blk.instructions = new
    return nc


def _make_in_maps(p, q, w):
    p = np.ascontiguousarray(p, dtype=np.float32)
    q = np.ascontiguousarray(q, dtype=np.float32)
    w = np.ascontiguousarray(w, dtype=np.float32)
    in_maps = []
    for c in range(NCORES):
        sl = slice(c * BP, (c + 1) * BP)
        in_maps.append({
            "pn": p[sl],
            "pt": np.ascontiguousarray(p[sl].transpose(0, 2, 1)),
            "qn": q[sl],
            "qt": np.ascontiguousarray(q[sl].transpose(0, 2, 1)),
            "w": w,
        })
    return in_maps


def run(p, q, w, trace=False):
    nc = legalize_waits(build_nc())
    res = run_bass_kernel_spmd(
        nc, _make_in_maps(p, q, w), list(range(NCORES)), trace=trace)
    out = np.concatenate([res.results[c]["g"] for c in range(NCORES)], axis=0)
    return out, res


def kernel(p, q, w):
    out, _ = run(p, q, w, trace=False)
    return out


# revision 34
# speedup vs baseline: 1.1145x; 1.0909x over previous
"""BiDAF attention kernel for trn2 (8 NeuronCores, pure data parallel).

v2: instruction-count-optimized.  S^T layout (LP on partitions) so both
softmaxes reduce along the free axis.  Aq = w^T q^T precomputed for all 16
batches in batched N=400 matmuls; h computed with reversed operands
(lhsT = eq column, rhs = p rows) with the normalizing sum fused in via a
ones-column appended to p; output rows assembled in SBUF so each LP-chunk
is a single contiguous-row DMA.
"""

from contextlib import ExitStack

import numpy as np

import concourse.bass as bass
import concourse.mybir as mybir
import concourse.tile as tile
from concourse.bass_utils import run_bass_kernel_spmd
from concourse.masks import make_identity

F32 = mybir.dt.float32
AX = mybir.AxisListType
ALU = mybir.AluOpType
ACTF = mybir.ActivationFunctionType

B, LP, LQ, H = 128, 400, 100, 256
NCORES = 8
BP = B // NCORES  # batches per core
ROWS = [(0, 128), (128, 128), (256, 128), (384, 16)]
BIG = float(np.float32(3.0e38))


def build_nc():
    nc = bass.Bass("TRN2", target_bir_lowering=False, debug=False)

    pn = nc.dram_tensor("pn", [BP, LP, H], F32, kind="ExternalInput")
    pt = nc.dram_tensor("pt", [BP, H, LP], F32, kind="ExternalInput")
    qn = nc.dram_tensor("qn", [BP, LQ, H], F32, kind="ExternalInput")
    qt = nc.dram_tensor("qt", [BP, H, LQ], F32, kind="ExternalInput")
    w = nc.dram_tensor("w", [H, H], F32, kind="ExternalInput")
    g = nc.dram_tensor("g", [BP, LP, 4 * H], F32, kind="ExternalOutput")

    with tile.TileContext(nc) as tc, ExitStack() as ctx:
        cpool = ctx.enter_context(tc.tile_pool(name="consts", bufs=1))
        wp = ctx.enter_context(tc.tile_pool(name="work", bufs=4))
        ppb = ctx.enter_context(tc.tile_pool(name="psb", bufs=2, space="PSUM"))
        pps = ctx.enter_context(tc.tile_pool(name="pss", bufs=2, space="PSUM"))
        ppu = ctx.enter_context(tc.tile_pool(name="psu", bufs=2, space="PSUM"))
        ppt = ctx.enter_context(tc.tile_pool(name="pst", bufs=2, space="PSUM"))

        # ---- constants ----
        ident = cpool.tile([128, 128], F32)
        make_identity(nc, ident[:])
        ones_r = cpool.tile([1, 128], F32)
        nc.vector.memset(ones_r[:], 1.0)
        Wt = cpool.tile([128, 2, H], F32)
        nc.sync.dma_start(Wt[:, 0, :], w[0:128, :])
        nc.sync.dma_start(Wt[:, 1, :], w[128:256, :])

        # ---- pre-pass: Aq = w^T q^T for ALL batches, batched 4-wide ----
        QtA = cpool.tile([128, 2, BP * LQ], F32)       # (h-part, kc, b*l)
        AqA = cpool.tile([128, 2, BP * LQ], F32)
        for gi in range(BP // 4):
            for kc in range(2):
                nc.sync.dma_start(
                    QtA[:, kc, gi * 400:(gi + 1) * 400].rearrange(
                        "p (b l) -> p b l", b=4),
                    qt[gi * 4:(gi + 1) * 4,
                       kc * 128:(kc + 1) * 128, :].rearrange(
                        "b p l -> p b l"))
        for gi in range(BP // 4):
            for ms in range(2):
                psAq = ppb.tile([128, 400], F32, tag="big")
                for kc in range(2):
                    nc.tensor.matmul(
                        psAq[:],
                        Wt[:, kc, ms * 128:(ms + 1) * 128],
                        QtA[:, kc, gi * 400:(gi + 1) * 400],
                        start=(kc == 0), stop=(kc == 1),
                    )
                nc.scalar.copy(AqA[:, ms, gi * 400:(gi + 1) * 400], psAq[:])

        for gi in range(BP // 4):
            grp = []
            NMN4 = wp.tile([128, 4], F32, tag="NMN4")
            for j in range(4):
                b = gi * 4 + j
                bq = b * LQ
                # ---------------- loads ----------------
                Pn = wp.tile([128, 4, H + 1], F32, tag="Pn")  # col 256: ones
                nc.vector.memset(Pn[:, :, H:H + 1], 1.0)
                nc.sync.dma_start(
                    Pn[0:128, 0:3, 0:H],
                    pn[b, 0:384, :].rearrange("(i r) h -> r i h", r=128))
                nc.sync.dma_start(Pn[0:16, 3, 0:H], pn[b, 384:400, :])
                Pt = wp.tile([128, 2, LP], F32, tag="Pt")
                nc.sync.dma_start(Pt[:],
                                  pt[b].rearrange("(k p) l -> p k l", p=128))
                Qn = wp.tile([128, H], F32, tag="Qn")
                nc.sync.dma_start(Qn[0:LQ, :], qn[b])

                # ---------------- S^T = p @ Aq  (LP, LQ) ----------------
                psS01 = pps.tile([128, 2, LQ], F32, tag="st")
                psS23 = pps.tile([128, 2, LQ], F32, tag="st")
                psSc = [psS01, psS01, psS23, psS23]
                for i, (off, r) in enumerate(ROWS):
                    for kc in range(2):
                        nc.tensor.matmul(
                            psSc[i][0:r, i % 2, :],
                            Pt[:, kc, off:off + r],
                            AqA[:, kc, bq:bq + LQ],
                            start=(kc == 0), stop=(kc == 1),
                        )

                # ---------------- C2Q softmax over LQ ----------------
                NM = wp.tile([128, 4], F32, tag="NM")
                RS = wp.tile([128, 4], F32, tag="RS")
                nc.vector.memset(NM[:], BIG)
                nc.vector.memset(RS[:], 1.0)
                for i, (off, r) in enumerate(ROWS):
                    nc.vector.tensor_reduce(
                        NM[0:r, i:i + 1], psSc[i][0:r, i % 2, :],
                        axis=AX.X, op=ALU.max, negate=True,
                    )
                E = wp.tile([128, 4, LQ], F32, tag="E")
                for i, (off, r) in enumerate(ROWS):
                    nc.scalar.activation(
                        E[0:r, i, :], psSc[i][0:r, i % 2, :], ACTF.Exp,
                        bias=NM[0:r, i:i + 1], accum_out=RS[0:r, i:i + 1],
                    )
                RCP = wp.tile([128, 4], F32, tag="RCP")
                nc.vector.reciprocal(RCP[:], RS[:])
                AT = wp.tile([128, 4, LQ], F32, tag="AT")
                for i, (off, r) in enumerate(ROWS):
                    nc.vector.tensor_scalar_mul(
                        AT[0:r, i, :], E[0:r, i, :], RCP[0:r, i:i + 1])

                # ---------------- transpose a^T -> a ----------------
                psAm = ppb.tile([128, 4, 128], F32, tag="big")
                for i, (off, r) in enumerate(ROWS):
                    nc.tensor.transpose(
                        psAm[0:LQ, i, 0:r], AT[0:r, i, :], ident[0:r, 0:r])
                Am = wp.tile([128, 4, 128], F32, tag="Am")
                nc.scalar.copy(Am[0:LQ, 0:3, :], psAm[0:LQ, 0:3, :])
                nc.scalar.copy(Am[0:LQ, 3, 0:16], psAm[0:LQ, 3, 0:16])

                # ---------------- U^T + output assembly ----------------
                Gt = wp.tile([128, 4, 4 * H], F32, tag="Gt")
                nc.vector.tensor_copy(Gt[0:128, 0:3, 0:H],
                                      Pn[0:128, 0:3, 0:H])
                nc.vector.tensor_copy(Gt[0:16, 3, 0:H], Pn[0:16, 3, 0:H])
                for i, (off, r) in enumerate(ROWS):
                    psU = ppu.tile([128, H], F32, tag="u")
                    nc.tensor.matmul(
                        psU[0:r, :], Am[0:LQ, i, 0:r], Qn[0:LQ, :],
                        start=True, stop=True,
                    )
                    nc.vector.tensor_tensor(
                        Gt[0:r, i, 2 * H:3 * H], Pn[0:r, i, 0:H],
                        psU[0:r, :], op=ALU.mult)

                nc.vector.tensor_reduce(NMN4[:, j:j + 1], NM[:], axis=AX.X,
                                        op=ALU.min)
                grp.append((b, Pn, Gt, NM))

            # ---------------- group Q2C glue (all 4 batches) ----------------
            psTB = ppt.tile([128, 257], F32, tag="tiny")
            nc.tensor.transpose(psTB[0:4, 0:128], NMN4[:], ident[:])
            ngmin4 = wp.tile([4, 1], F32, tag="ngmin4")   # -gmax per batch
            nc.vector.tensor_reduce(ngmin4[:], psTB[0:4, 0:128], axis=AX.X,
                                    op=ALU.min)
            nc.tensor.transpose(psTB[0:1, 132:136], ngmin4[:],
                                ident[0:4, 0:4])
            ngr = wp.tile([1, 4], F32, tag="ngr")
            nc.scalar.copy(ngr[:], psTB[0:1, 132:136])
            psB4 = ppt.tile([128, 257], F32, tag="tiny")
            nc.tensor.matmul(psB4[0:128, 0:4], ones_r[:], ngr[:],
                             start=True, stop=True)
            nb4 = wp.tile([128, 4], F32, tag="nb4")
            nc.scalar.copy(nb4[:], psB4[0:128, 0:4])

            for j in range(4):
                b, Pn, Gt, NM = grp[j]
                EQ = wp.tile([128, 4], F32, tag="EQ")
                nc.scalar.activation(EQ[:], NM[:], ACTF.Exp,
                                     bias=nb4[:, j:j + 1], scale=-1.0)
                psHr = ppt.tile([128, 257], F32, tag="tiny")
                for i, (off, r) in enumerate(ROWS):
                    nc.tensor.matmul(
                        psHr[0:1, :],
                        EQ[0:r, i:i + 1],
                        Pn[0:r, i, 0:H + 1],
                        start=(i == 0), stop=(i == 3),
                    )
                rq = wp.tile([1, 1], F32, tag="rq")
                nc.vector.reciprocal(rq[:], psHr[0:1, H:H + 1])
                hrow = wp.tile([1, H], F32, tag="hrow")
                nc.scalar.mul(hrow[:], psHr[0:1, 0:H], rq[:])
                psHt = ppb.tile([128, H], F32, tag="big")
                nc.tensor.matmul(psHt[:], ones_r[:], hrow[:], start=True,
                                 stop=True)
                HtS = wp.tile([128, H], F32, tag="HtS")
                nc.vector.tensor_copy(HtS[:], psHt[:])
                for i, (off, r) in enumerate(ROWS):
                    nc.vector.tensor_copy(Gt[0:r, i, H:2 * H], HtS[0:r, :])
                    nc.gpsimd.tensor_tensor(
                        Gt[0:r, i, 3 * H:4 * H], Pn[0:r, i, 0:H],
                        HtS[0:r, :], op=ALU.mult)
                for i, (off, r) in enumerate(ROWS):
                    nc.sync.dma_start(g[b, off:off + r, :], Gt[0:r, i, :])

    return nc


def legalize_waits(nc):
    """Split multi-wait instructions into single-wait NoOps + instruction.

    The TPB ISA has exactly one (wait, update) EVENTS slot per 64B
    instruction; this walrus build refuses instructions with more than one
    sync wait ("Too many sync wait commands").  Tile's scheduler emits
    vector-clock waits freely, so legalize here: excess waits move onto
    engine-queue NoOps placed immediately before the instruction.
    """
    counter = 0
    for f in nc.m.functions:
        for blk in f.blocks:
            new = []
            for inst in blk.instructions:
                si = getattr(inst, "sync_info", None)
                if si is not None and len(si.on_wait) > 1:
                    waits = list(si.on_wait)
                    assert len(si.on_update) <= 1, inst
                    for wt in waits[:-1]:
                        counter += 1
                        new.append(mybir.InstNoOp(
                            name=f"I-waitnop-{counter}",
                            engine=inst.engine,
                            sync_info=mybir.SyncInfo(on_wait=[wt],
                                                     on_update=[]),
                        ))
                    inst.sync_info = mybir.SyncInfo(
                        on_wait=[waits[-1]], on_update=list(si.on_update))
                new.append(inst)
            blk.instructions = new
    return nc


def _make_in_maps(p, q, w):
    p = np.ascontiguousarray(p, dtype=np.float32)
    q = np.ascontiguousarray(q, dtype=np.float32)
    w = np.ascontiguousarray(w, dtype=np.float32)
    in_maps = []
    for c in range(NCORES):
        sl = slice(c * BP, (c + 1) * BP)
        in_maps.append({
            "pn": p[sl],
            "pt": np.ascontiguousarray(p[sl].transpose(0, 2, 1)),
            "qn": q[sl],
            "qt": np.ascontiguousarray(q[sl].transpose(0, 2, 1)),
            "w": w,
        })
    return in_maps


def run(p, q, w, trace=False):
    nc = legalize_waits(build_nc())
    res = run_bass_kernel_spmd(
        nc, _make_in_maps(p, q, w), list(range(NCORES)), trace=trace)
    out = np.concatenate([res.results[c]["g"] for c in range(NCORES)], axis=0)
    return out, res


def kernel(p, q, w):
    out, _ = run(p, q, w, trace=False)
    return out
